# revision 4
# baseline (speedup 1.0000x reference)
"""LGCN encoder kernel for 8 Trainium2 cores.

Strategy: rows (nodes) sharded across 8 cores; ego embeddings replicated.
The two SpMMs (A@ego, R@ego) are computed as one-hot matmuls: edges sorted by
(output tile of 128 rows, 32k col-window), gathered 128-at-a-time via the
custom dma_gather ucode (int16 window-relative indices), multiplied by a
val-weighted one-hot built on DVE (iota is_equal localrow * val), accumulated
in PSUM by the tensor engine. mean = 0.25*ego + 0.75*(A@ego) on device.
Host does layout only: shard/sort/pack inputs, stack outputs.
"""
import os
from contextlib import ExitStack

import numpy as np
import ml_dtypes

U, I, D = 100000, 50000, 64
N = U + I                 # 150000
NCORES = 8
NPAD = 150528             # 8 * 18816
RPC = NPAD // NCORES      # 18816 rows per core
T = RPC // 128            # 147 tiles per core
GT = 8                    # tiles per group
G = (T + GT - 1) // GT    # 19 groups (last has 3 tiles)
WIN = 32768
NW = (NPAD + WIN - 1) // WIN  # 5 col windows
EP = 128                  # padded bf16 row elems (256 B)

LAST_EXEC_NS = None


def _pack_idxs(flat_slots):
    """flat_slots: [n*128] int16 slot-ordered idx list -> [128, n*8] wrapped."""
    num = flat_slots.shape[0]
    ncols = num // 16
    blk = flat_slots.reshape(ncols, 16).T            # [16, ncols]
    return np.tile(blk, (8, 1))                      # [128, ncols]


def _preprocess_adj(rows, cols, vals):
    """Returns per-core packed meta/idx arrays + shared chunk schedule."""
    rows = rows.astype(np.int64)
    cols = cols.astype(np.int64)
    tg = rows // 128                                  # global tile 0..1175
    w = cols // WIN                                   # window 0..4
    order = np.lexsort((w, tg))
    r, c, v, tg, w = rows[order], cols[order], vals[order], tg[order], w[order]
    core = tg // T
    bid = tg * NW + w                                 # bucket id
    NB = NCORES * T * NW
    cnt = np.bincount(bid, minlength=NB)
    cnt_ct = cnt.reshape(NCORES, T * NW)
    Kw = -(-cnt_ct.max(axis=0) // 128)                # [T*NW] shared chunk counts
    Kw = Kw.reshape(T, NW)                            # Kw[t, w]

    # per-group chunk schedule (w-major): for w, for t in group, Kw[t,w] chunks
    # bucketcol[t, w] = chunk column offset within group g = t//GT
    bucketcol = np.zeros((T, NW), dtype=np.int64)
    woff = np.zeros((G, NW), dtype=np.int64)          # chunk offset of w-run in group
    Cg = np.zeros(G, dtype=np.int64)
    for g in range(G):
        ts = range(g * GT, min((g + 1) * GT, T))
        pos = 0
        for ww in range(NW):
            woff[g, ww] = pos
            for t in ts:
                bucketcol[t, ww] = pos
                pos += Kw[t, ww]
        Cg[g] = pos
    Cmax = int(Cg.max())

    # edge slot assignment
    starts = np.zeros(NB, dtype=np.int64)
    np.cumsum(cnt[:-1], out=starts[1:])
    # per-core rank: edges sorted by bid; rank within (core-specific bucket)
    rank = np.arange(len(r)) - starts[bid]
    # but starts are global; edges of one bid can span... bid includes core via
    # tg, so each (core) has distinct bids. rank is within-bucket ✓
    k = rank // 128
    lane = rank % 128
    tl = tg % T                                       # tile within core
    gid = tl // GT
    colg = bucketcol[tl, w] + k                       # chunk col within group
    localrow = (r % 128).astype(np.float32)
    relcol = (c - w * WIN).astype(np.int16)

    meta = np.zeros((NCORES, G, 128, 2 * Cmax), dtype=np.float32)
    flat = meta.reshape(-1)
    mbase = ((core * G + gid) * 128 + lane) * (2 * Cmax)
    flat[mbase + 2 * colg] = localrow
    flat[mbase + 2 * colg + 1] = v.astype(np.float32)

    # slot-order idx lists per (core, g): slot = colg*128 + lane
    sl = np.zeros((NCORES, G, Cmax * 128), dtype=np.int16)
    slf = sl.reshape(-1)
    sbase = (core * G + gid) * (Cmax * 128)
    slf[sbase + colg * 128 + lane] = relcol

    idxs = np.zeros((NCORES, G, 128, 8 * Cmax), dtype=np.int16)
    for cc in range(NCORES):
        for g in range(G):
            if Cg[g]:
                idxs[cc, g, :, : 8 * Cg[g]] = _pack_idxs(sl[cc, g, : Cg[g] * 128])

    return meta, idxs, Kw, woff, Cg, Cmax


_CACHE = {}


def _build_program(Kw_a, woff_a, Cg_a, Cmax_a, Kw_r, woff_r, Cg_r, Cmax_r):
    import concourse.tile as tile
    from concourse import bacc, mybir
    from concourse import library_config

    nc = bacc.Bacc("TRN2", target_bir_lowering=False, debug=False,
                   num_devices=NCORES)
    dt = mybir.dt
    ego16_t = nc.dram_tensor("ego16", (NPAD, EP), dt.bfloat16, kind="ExternalInput")
    egof_t = nc.dram_tensor("egof", (G, 128, GT * D), dt.float32, kind="ExternalInput")
    iota_t = nc.dram_tensor("iota", (128, 128), dt.bfloat16, kind="ExternalInput")
    meta_ts = [nc.dram_tensor(f"meta{a}", (G, 128, 2 * cm), dt.float32,
                              kind="ExternalInput")
               for a, cm in ((0, Cmax_a), (1, Cmax_r))]
    idx_ts = [nc.dram_tensor(f"idxs{a}", (G, 128, 8 * cm), dt.int16,
                             kind="ExternalInput")
              for a, cm in ((0, Cmax_a), (1, Cmax_r))]
    y_ts = [nc.dram_tensor(nm, (G, 128, GT * D), dt.float32, kind="ExternalOutput")
            for nm in ("ya", "yr")]
    mn_t = nc.dram_tensor("mn", (G, 128, GT * D), dt.float32, kind="ExternalOutput")

    adj = [(Kw_a, woff_a, Cg_a), (Kw_r, woff_r, Cg_r)]

    with tile.TileContext(nc) as tc:
        with ExitStack() as ctx:
            cst = ctx.enter_context(tc.tile_pool(name="cst", bufs=1))
            mp = ctx.enter_context(tc.tile_pool(name="mp", bufs=3))
            ip = ctx.enter_context(tc.tile_pool(name="ip", bufs=3))
            ep_ = ctx.enter_context(tc.tile_pool(name="ep", bufs=2))
            gp = ctx.enter_context(tc.tile_pool(name="gp", bufs=2 * NW))
            ohp = ctx.enter_context(tc.tile_pool(name="ohp", bufs=6))
            psp = ctx.enter_context(tc.tile_pool(name="psp", bufs=8, space="PSUM"))
            yp = ctx.enter_context(tc.tile_pool(name="yp", bufs=3))
            tp = ctx.enter_context(tc.tile_pool(name="tp", bufs=4))

            nc.gpsimd.load_library(library_config.mlp)
            iota_sb = cst.tile([128, 128], dt.bfloat16, tag="iota")
            nc.sync.dma_start(iota_sb[:], iota_t.ap()[:, :])

            for a in (0, 1):
                Kw, woff, Cg = adj[a]
                for g in range(G):
                    C = int(Cg[g])
                    if C == 0:
                        continue
                    nt = min(GT, T - g * GT)
                    idx_sb = ip.tile([128, 8 * C], dt.int16, tag="idx")
                    nc.sync.dma_start(idx_sb[:], idx_ts[a].ap()[g, :, : 8 * C])
                    meta_sb = mp.tile([128, 2 * C], dt.float32, tag="meta")
                    nc.sync.dma_start(meta_sb[:], meta_ts[a].ap()[g, :, : 2 * C])
                    if a == 0:
                        ego_sb = ep_.tile([128, nt * D], dt.float32, tag="ego")
                        nc.sync.dma_start(ego_sb[:], egof_t.ap()[g, :, : nt * D])

                    gths = {}
                    for ww in range(NW):
                        kr = int(Kw[g * GT : g * GT + nt, ww].sum())
                        if kr == 0:
                            continue
                        o = int(woff[g, ww])
                        gth = gp.tile([128, kr, EP], dt.bfloat16, tag="gth")
                        w0 = ww * WIN
                        nc.gpsimd.dma_gather(
                            out_ap=gth[:],
                            in_ap=ego16_t.ap()[w0 : min(w0 + WIN, NPAD), :],
                            idxs_ap=idx_sb[:, 8 * o : 8 * (o + kr)],
                            num_idxs=kr * 128,
                            num_idxs_reg=kr * 128,
                            elem_size=EP,
                            single_packet=False,
                        )
                        gths[ww] = (gth, o)

                    # per-tile psum accumulation across w-major chunk order
                    pst = {}
                    done = np.zeros(nt, dtype=np.int64)
                    tot = Kw[g * GT : g * GT + nt, :].sum(axis=1)
                    for ww in range(NW):
                        if ww not in gths:
                            continue
                        gth, o = gths[ww]
                        ci = 0
                        for ti in range(nt):
                            t = g * GT + ti
                            for _k in range(int(Kw[t, ww])):
                                col = int(woff[g, ww]) + ci
                                oh = ohp.tile([128, 128], dt.bfloat16, tag="oh")
                                nc.vector.tensor_scalar(
                                    out=oh[:], in0=iota_sb[:],
                                    scalar1=meta_sb[:, 2 * col : 2 * col + 1],
                                    scalar2=meta_sb[:, 2 * col + 1 : 2 * col + 2],
                                    op0=mybir.AluOpType.is_equal,
                                    op1=mybir.AluOpType.mult,
                                )
                                if ti not in pst:
                                    pst[ti] = psp.tile([128, D], dt.float32,
                                                       name="ps", tag="ps")
                                nc.tensor.matmul(
                                    out=pst[ti][:], lhsT=oh[:],
                                    rhs=gth[:, ci, 0:D],
                                    start=(done[ti] == 0),
                                    stop=(done[ti] == tot[ti] - 1),
                                )
                                done[ti] += 1
                                ci += 1

                    ygrp = yp.tile([128, nt * D], dt.float32, tag="ygrp")
                    if a == 0:
                        mgrp = yp.tile([128, nt * D], dt.float32, tag="mgrp")
                    for ti in range(nt):
                        sl_ = slice(ti * D, (ti + 1) * D)
                        if ti in pst:
                            nc.scalar.copy(ygrp[:, sl_], pst[ti][:])
                            if a == 0:
                                tmp = tp.tile([128, D], dt.float32, tag="tmp")
                                nc.vector.tensor_scalar(
                                    out=tmp[:], in0=pst[ti][:], scalar1=0.75,
                                    scalar2=None, op0=mybir.AluOpType.mult)
                                nc.vector.tensor_add(
                                    out=mgrp[:, sl_], in0=tmp[:],
                                    in1=ego_sb[:, sl_])
                        else:
                            nc.vector.memset(ygrp[:, sl_], 0.0)
                            if a == 0:
                                nc.vector.tensor_copy(out=mgrp[:, sl_],
                                                      in_=ego_sb[:, sl_])
                    nc.sync.dma_start(y_ts[a].ap()[g, :, : nt * D], ygrp[:])
                    if a == 0:
                        nc.sync.dma_start(mn_t.ap()[g, :, : nt * D], mgrp[:])

    nc.compile()
    return nc


def _unpack_y(arr):
    """[G,128,GT*D] device layout -> [RPC, D]."""
    y = arr.reshape(G, 128, GT, D).transpose(0, 2, 1, 3).reshape(G * GT * 128, D)
    return y[:RPC]


def kernel(user_emb, item_emb, adj_rows, adj_cols, adj_vals,
           radj_rows, radj_cols, radj_vals):
    global LAST_EXEC_NS
    from concourse.bass_utils import run_bass_kernel_spmd

    ego = np.concatenate([np.asarray(user_emb, np.float32),
                          np.asarray(item_emb, np.float32)], axis=0)
    egop = np.zeros((NPAD, D), dtype=np.float32)
    egop[:N] = ego
    ego16 = np.zeros((NPAD, EP), dtype=ml_dtypes.bfloat16)
    ego16[:, :D] = egop.astype(ml_dtypes.bfloat16)

    meta_a, idx_a, Kw_a, woff_a, Cg_a, Cmax_a = _preprocess_adj(
        np.asarray(adj_rows), np.asarray(adj_cols), np.asarray(adj_vals))
    meta_r, idx_r, Kw_r, woff_r, Cg_r, Cmax_r = _preprocess_adj(
        np.asarray(radj_rows), np.asarray(radj_cols), np.asarray(radj_vals))

    key = (tuple(Kw_a.ravel()), tuple(Kw_r.ravel()))
    if key not in _CACHE:
        _CACHE.clear()
        _CACHE[key] = _build_program(Kw_a, woff_a, Cg_a, Cmax_a,
                                     Kw_r, woff_r, Cg_r, Cmax_r)
    nc = _CACHE[key]

    iota_np = np.asarray(
        np.broadcast_to(np.arange(128, dtype=np.float32), (128, 128))
    ).astype(ml_dtypes.bfloat16)

    in_maps = []
    for cc in range(NCORES):
        blk = egop[cc * RPC : (cc + 1) * RPC] * 0.25
        egof = np.zeros((G, 128, GT * D), dtype=np.float32)
        full = blk.reshape(T, 128, D)
        for g in range(G):
            nt = min(GT, T - g * GT)
            egof[g, :, : nt * D] = (
                full[g * GT : g * GT + nt].transpose(1, 0, 2).reshape(128, nt * D))
        in_maps.append({
            "ego16": np.asarray(ego16),
            "egof": egof,
            "iota": iota_np,
            "meta0": meta_a[cc], "meta1": meta_r[cc],
            "idxs0": idx_a[cc], "idxs1": idx_r[cc],
        })

    import time as _time
    t0 = _time.monotonic()
    res = run_bass_kernel_spmd(nc, in_maps, core_ids=list(range(NCORES)))
    LAST_EXEC_NS = res.exec_time_ns
    if LAST_EXEC_NS is None:
        LAST_EXEC_NS = int((_time.monotonic() - t0) * 1e9)  # wall upper bound

    YA = np.empty((NPAD, D), dtype=np.float32)
    YR = np.empty((NPAD, D), dtype=np.float32)
    MN = np.empty((NPAD, D), dtype=np.float32)
    for cc in range(NCORES):
        s = slice(cc * RPC, (cc + 1) * RPC)
        YA[s] = _unpack_y(res.results[cc]["ya"])
        YR[s] = _unpack_y(res.results[cc]["yr"])
        MN[s] = _unpack_y(res.results[cc]["mn"])

    mean = MN[:N]
    user_all = np.ascontiguousarray(mean[:U])
    item_all = np.ascontiguousarray(mean[U:N])
    stacked = np.empty((N, 4, D), dtype=np.float32)
    stacked[:, 0] = ego
    stacked[:, 1] = stacked[:, 2] = stacked[:, 3] = YA[:N]
    path_stacked = np.empty((N, 4, D), dtype=np.float32)
    path_stacked[:, 0] = ego
    path_stacked[:, 1] = path_stacked[:, 2] = path_stacked[:, 3] = YR[:N]
    return (user_all, item_all, stacked, path_stacked)


# revision 5
# speedup vs baseline: 1.2948x; 1.2948x over previous
"""LGCN encoder kernel for 8 Trainium2 cores.

Strategy: rows (nodes) sharded across 8 cores; ego embeddings replicated.
The two SpMMs (A@ego, R@ego) are computed as one-hot matmuls: edges sorted by
(output tile of 128 rows, 32k col-window), gathered 128-at-a-time via the
custom dma_gather ucode (int16 window-relative indices), multiplied by a
val-weighted one-hot built on DVE (iota is_equal localrow * val), accumulated
in PSUM by the tensor engine. mean = 0.25*ego + 0.75*(A@ego) on device.
Host does layout only: shard/sort/pack inputs, stack outputs.
"""
import os
from contextlib import ExitStack

import numpy as np
import ml_dtypes

U, I, D = 100000, 50000, 64
N = U + I                 # 150000
NCORES = 8
NPAD = 150528             # 8 * 18816
RPC = NPAD // NCORES      # 18816 rows per core
T = RPC // 128            # 147 tiles per core
GT = 8                    # tiles per group
G = (T + GT - 1) // GT    # 19 groups (last has 3 tiles)
WIN = 32768
NW = (NPAD + WIN - 1) // WIN  # 5 col windows
EP = 128                  # padded bf16 row elems (256 B)

LAST_EXEC_NS = None


def _pack_idxs(flat_slots):
    """flat_slots: [n*128] int16 slot-ordered idx list -> [128, n*8] wrapped."""
    num = flat_slots.shape[0]
    ncols = num // 16
    blk = flat_slots.reshape(ncols, 16).T            # [16, ncols]
    return np.tile(blk, (8, 1))                      # [128, ncols]


def _preprocess_adj(rows, cols, vals):
    """Returns per-core packed meta/idx arrays + shared chunk schedule."""
    rows = rows.astype(np.int64)
    cols = cols.astype(np.int64)
    tg = rows // 128                                  # global tile 0..1175
    w = cols // WIN                                   # window 0..4
    order = np.lexsort((w, tg))
    r, c, v, tg, w = rows[order], cols[order], vals[order], tg[order], w[order]
    core = tg // T
    bid = tg * NW + w                                 # bucket id
    NB = NCORES * T * NW
    cnt = np.bincount(bid, minlength=NB)
    cnt_ct = cnt.reshape(NCORES, T * NW)
    Kw = -(-cnt_ct.max(axis=0) // 128)                # [T*NW] shared chunk counts
    Kw = Kw.reshape(T, NW)                            # Kw[t, w]

    # per-group chunk schedule (w-major): for w, for t in group, Kw[t,w] chunks
    # bucketcol[t, w] = chunk column offset within group g = t//GT
    bucketcol = np.zeros((T, NW), dtype=np.int64)
    woff = np.zeros((G, NW), dtype=np.int64)          # chunk offset of w-run in group
    Cg = np.zeros(G, dtype=np.int64)
    for g in range(G):
        ts = range(g * GT, min((g + 1) * GT, T))
        pos = 0
        for ww in range(NW):
            woff[g, ww] = pos
            for t in ts:
                bucketcol[t, ww] = pos
                pos += Kw[t, ww]
        Cg[g] = pos
    Cmax = int(Cg.max())

    # edge slot assignment
    starts = np.zeros(NB, dtype=np.int64)
    np.cumsum(cnt[:-1], out=starts[1:])
    # per-core rank: edges sorted by bid; rank within (core-specific bucket)
    rank = np.arange(len(r)) - starts[bid]
    # but starts are global; edges of one bid can span... bid includes core via
    # tg, so each (core) has distinct bids. rank is within-bucket ✓
    k = rank // 128
    lane = rank % 128
    tl = tg % T                                       # tile within core
    gid = tl // GT
    colg = bucketcol[tl, w] + k                       # chunk col within group
    localrow = (r % 128).astype(np.float32)
    relcol = (c - w * WIN).astype(np.int16)

    meta = np.zeros((NCORES, G, 128, 2 * Cmax), dtype=np.float32)
    flat = meta.reshape(-1)
    mbase = ((core * G + gid) * 128 + lane) * (2 * Cmax)
    flat[mbase + 2 * colg] = localrow
    flat[mbase + 2 * colg + 1] = v.astype(np.float32)

    # slot-order idx lists per (core, g): slot = colg*128 + lane
    sl = np.zeros((NCORES, G, Cmax * 128), dtype=np.int16)
    slf = sl.reshape(-1)
    sbase = (core * G + gid) * (Cmax * 128)
    slf[sbase + colg * 128 + lane] = relcol

    idxs = np.zeros((NCORES, G, 128, 8 * Cmax), dtype=np.int16)
    for cc in range(NCORES):
        for g in range(G):
            if Cg[g]:
                idxs[cc, g, :, : 8 * Cg[g]] = _pack_idxs(sl[cc, g, : Cg[g] * 128])

    return meta, idxs, Kw, woff, Cg, Cmax


_CACHE = {}


def _build_program(Kw_a, woff_a, Cg_a, Cmax_a, Kw_r, woff_r, Cg_r, Cmax_r):
    import concourse.tile as tile
    from concourse import bacc, mybir
    from concourse import library_config

    nc = bacc.Bacc("TRN2", target_bir_lowering=False, debug=False,
                   num_devices=NCORES)
    dt = mybir.dt
    ego16_t = nc.dram_tensor("ego16", (NPAD, EP), dt.bfloat16, kind="ExternalInput")
    egof_t = nc.dram_tensor("egof", (G, 128, GT * D), dt.float32, kind="ExternalInput")
    iota_t = nc.dram_tensor("iota", (128, 128), dt.bfloat16, kind="ExternalInput")
    meta_ts = [nc.dram_tensor(f"meta{a}", (G, 128, 2 * cm), dt.float32,
                              kind="ExternalInput")
               for a, cm in ((0, Cmax_a), (1, Cmax_r))]
    idx_ts = [nc.dram_tensor(f"idxs{a}", (G, 128, 8 * cm), dt.int16,
                             kind="ExternalInput")
              for a, cm in ((0, Cmax_a), (1, Cmax_r))]
    y_ts = [nc.dram_tensor(nm, (G, 128, GT * D), dt.float32, kind="ExternalOutput")
            for nm in ("ya", "yr")]
    mn_t = nc.dram_tensor("mn", (G, 128, GT * D), dt.float32, kind="ExternalOutput")

    adj = [(Kw_a, woff_a, Cg_a), (Kw_r, woff_r, Cg_r)]

    with tile.TileContext(nc) as tc:
        with ExitStack() as ctx:
            cst = ctx.enter_context(tc.tile_pool(name="cst", bufs=1))
            mp = ctx.enter_context(tc.tile_pool(name="mp", bufs=3))
            ip = ctx.enter_context(tc.tile_pool(name="ip", bufs=3))
            ep_ = ctx.enter_context(tc.tile_pool(name="ep", bufs=2))
            gp = ctx.enter_context(tc.tile_pool(name="gp", bufs=3 * NW))
            ohp = ctx.enter_context(tc.tile_pool(name="ohp", bufs=12))
            psp = ctx.enter_context(tc.tile_pool(name="psp", bufs=8, space="PSUM"))
            yp = ctx.enter_context(tc.tile_pool(name="yp", bufs=4))
            tp = ctx.enter_context(tc.tile_pool(name="tp", bufs=4))

            nc.gpsimd.load_library(library_config.mlp)
            iota_sb = cst.tile([128, 128], dt.bfloat16, tag="iota")
            nc.sync.dma_start(iota_sb[:], iota_t.ap()[:, :])

            for a in (0, 1):
                Kw, woff, Cg = adj[a]
                for g in range(G):
                    C = int(Cg[g])
                    if C == 0:
                        continue
                    nt = min(GT, T - g * GT)
                    idx_sb = ip.tile([128, 8 * C], dt.int16, tag="idx")
                    nc.sync.dma_start(idx_sb[:], idx_ts[a].ap()[g, :, : 8 * C])
                    meta_sb = mp.tile([128, 2 * C], dt.float32, tag="meta")
                    nc.sync.dma_start(meta_sb[:], meta_ts[a].ap()[g, :, : 2 * C])
                    if a == 0:
                        ego_sb = ep_.tile([128, nt * D], dt.float32, tag="ego")
                        nc.sync.dma_start(ego_sb[:], egof_t.ap()[g, :, : nt * D])

                    gths = {}
                    for ww in range(NW):
                        kr = int(Kw[g * GT : g * GT + nt, ww].sum())
                        if kr == 0:
                            continue
                        o = int(woff[g, ww])
                        gth = gp.tile([128, kr, EP], dt.bfloat16, tag="gth")
                        w0 = ww * WIN
                        nc.gpsimd.dma_gather(
                            out_ap=gth[:],
                            in_ap=ego16_t.ap()[w0 : min(w0 + WIN, NPAD), :],
                            idxs_ap=idx_sb[:, 8 * o : 8 * (o + kr)],
                            num_idxs=kr * 128,
                            num_idxs_reg=kr * 128,
                            elem_size=EP,
                            single_packet=False,
                        )
                        gths[ww] = (gth, o)

                    # per-tile psum accumulation across w-major chunk order
                    pst = {}
                    done = np.zeros(nt, dtype=np.int64)
                    tot = Kw[g * GT : g * GT + nt, :].sum(axis=1)
                    for ww in range(NW):
                        if ww not in gths:
                            continue
                        gth, o = gths[ww]
                        ci = 0
                        for ti in range(nt):
                            t = g * GT + ti
                            for _k in range(int(Kw[t, ww])):
                                col = int(woff[g, ww]) + ci
                                oh = ohp.tile([128, 128], dt.bfloat16, tag="oh")
                                nc.vector.tensor_scalar(
                                    out=oh[:], in0=iota_sb[:],
                                    scalar1=meta_sb[:, 2 * col : 2 * col + 1],
                                    scalar2=meta_sb[:, 2 * col + 1 : 2 * col + 2],
                                    op0=mybir.AluOpType.is_equal,
                                    op1=mybir.AluOpType.mult,
                                )
                                if ti not in pst:
                                    pst[ti] = psp.tile([128, D], dt.float32,
                                                       name="ps", tag="ps")
                                nc.tensor.matmul(
                                    out=pst[ti][:], lhsT=oh[:],
                                    rhs=gth[:, ci, 0:D],
                                    start=(done[ti] == 0),
                                    stop=(done[ti] == tot[ti] - 1),
                                )
                                done[ti] += 1
                                ci += 1

                    ygrp = yp.tile([128, nt * D], dt.float32, tag="ygrp")
                    if a == 0:
                        mgrp = yp.tile([128, nt * D], dt.float32, tag="mgrp")
                    for ti in range(nt):
                        sl_ = slice(ti * D, (ti + 1) * D)
                        if ti in pst:
                            nc.scalar.copy(ygrp[:, sl_], pst[ti][:])
                            if a == 0:
                                tmp = tp.tile([128, D], dt.float32, tag="tmp")
                                nc.scalar.mul(tmp[:], pst[ti][:], 0.75)
                                nc.vector.tensor_add(
                                    out=mgrp[:, sl_], in0=tmp[:],
                                    in1=ego_sb[:, sl_])
                        else:
                            nc.vector.memset(ygrp[:, sl_], 0.0)
                            if a == 0:
                                nc.vector.tensor_copy(out=mgrp[:, sl_],
                                                      in_=ego_sb[:, sl_])
                    nc.sync.dma_start(y_ts[a].ap()[g, :, : nt * D], ygrp[:])
                    if a == 0:
                        nc.sync.dma_start(mn_t.ap()[g, :, : nt * D], mgrp[:])

    nc.compile()
    return nc


def _unpack_y(arr):
    """[G,128,GT*D] device layout -> [RPC, D]."""
    y = arr.reshape(G, 128, GT, D).transpose(0, 2, 1, 3).reshape(G * GT * 128, D)
    return y[:RPC]


def kernel(user_emb, item_emb, adj_rows, adj_cols, adj_vals,
           radj_rows, radj_cols, radj_vals):
    global LAST_EXEC_NS
    from concourse.bass_utils import run_bass_kernel_spmd

    ego = np.concatenate([np.asarray(user_emb, np.float32),
                          np.asarray(item_emb, np.float32)], axis=0)
    egop = np.zeros((NPAD, D), dtype=np.float32)
    egop[:N] = ego
    ego16 = np.zeros((NPAD, EP), dtype=ml_dtypes.bfloat16)
    ego16[:, :D] = egop.astype(ml_dtypes.bfloat16)

    meta_a, idx_a, Kw_a, woff_a, Cg_a, Cmax_a = _preprocess_adj(
        np.asarray(adj_rows), np.asarray(adj_cols), np.asarray(adj_vals))
    meta_r, idx_r, Kw_r, woff_r, Cg_r, Cmax_r = _preprocess_adj(
        np.asarray(radj_rows), np.asarray(radj_cols), np.asarray(radj_vals))

    key = (tuple(Kw_a.ravel()), tuple(Kw_r.ravel()))
    if key not in _CACHE:
        _CACHE.clear()
        _CACHE[key] = _build_program(Kw_a, woff_a, Cg_a, Cmax_a,
                                     Kw_r, woff_r, Cg_r, Cmax_r)
    nc = _CACHE[key]

    iota_np = np.asarray(
        np.broadcast_to(np.arange(128, dtype=np.float32), (128, 128))
    ).astype(ml_dtypes.bfloat16)

    in_maps = []
    for cc in range(NCORES):
        blk = egop[cc * RPC : (cc + 1) * RPC] * 0.25
        egof = np.zeros((G, 128, GT * D), dtype=np.float32)
        full = blk.reshape(T, 128, D)
        for g in range(G):
            nt = min(GT, T - g * GT)
            egof[g, :, : nt * D] = (
                full[g * GT : g * GT + nt].transpose(1, 0, 2).reshape(128, nt * D))
        in_maps.append({
            "ego16": np.asarray(ego16),
            "egof": egof,
            "iota": iota_np,
            "meta0": meta_a[cc], "meta1": meta_r[cc],
            "idxs0": idx_a[cc], "idxs1": idx_r[cc],
        })

    import time as _time
    t0 = _time.monotonic()
    res = run_bass_kernel_spmd(nc, in_maps, core_ids=list(range(NCORES)))
    LAST_EXEC_NS = res.exec_time_ns
    if LAST_EXEC_NS is None:
        LAST_EXEC_NS = int((_time.monotonic() - t0) * 1e9)  # wall upper bound

    YA = np.empty((NPAD, D), dtype=np.float32)
    YR = np.empty((NPAD, D), dtype=np.float32)
    MN = np.empty((NPAD, D), dtype=np.float32)
    for cc in range(NCORES):
        s = slice(cc * RPC, (cc + 1) * RPC)
        YA[s] = _unpack_y(res.results[cc]["ya"])
        YR[s] = _unpack_y(res.results[cc]["yr"])
        MN[s] = _unpack_y(res.results[cc]["mn"])

    mean = MN[:N]
    user_all = np.ascontiguousarray(mean[:U])
    item_all = np.ascontiguousarray(mean[U:N])
    stacked = np.empty((N, 4, D), dtype=np.float32)
    stacked[:, 0] = ego
    stacked[:, 1] = stacked[:, 2] = stacked[:, 3] = YA[:N]
    path_stacked = np.empty((N, 4, D), dtype=np.float32)
    path_stacked[:, 0] = ego
    path_stacked[:, 1] = path_stacked[:, 2] = path_stacked[:, 3] = YR[:N]
    return (user_all, item_all, stacked, path_stacked)
